# revision 31
# baseline (speedup 1.0000x reference)
"""Trainium2 Bass kernel for nn_AttentionPixelClassifier.

Math: the query sits at the RoPE origin, so its rotation is the identity and
every downstream op is linear in the trig values of the key positions.  The
whole pipeline (k-projection, RoPE, q.kT, head-combining linear) collapses to

    logits[c, s] = sum_j t[j, s] * (G[c]^T x[c])[j, s] + bo

where G[c] = Wk @ M[c] is a per-instance [ES, 8] matrix folded on the host
from (q[c], Wo, Wk), and t is an [8, S] table of cos/sin terms of the pixel
positions.  On-device work per core (one instance per core, data-parallel
over C): a [256 -> 8] matmul over all 65536 pixels, an elementwise multiply
by t, and an 8-row reduction.  The kernel is HBM-bound: 64 MiB of x per core.

Device schedule: 512-column sub-tiles, grouped in quads.  Each sub-tile u
does two K=128 matmuls (G halves) into a [8, 512] PSUM bank, then a DVE
multiply by t writes the product into rows [32q, 32q+8) of a shared
[128, 512] SBUF tile (32-aligned partition bases are required by BIR).  One
K=128 block-ones matmul then reduces all four sub-tiles at once into a
[4, 512] PSUM tile, which is copied (DVE, to keep matmul sync-waits
mergeable) and DMA'd out.  fp32 matmuls stream at ~2 cycles/column, so the
packed reduce keeps TensorE off the critical path; HBM DMA is the floor.
"""

import numpy as np

C, ES, H, W = 8, 256, 256, 256
NH, HD = 4, 8
S = H * W            # 65536 pixels
NCORES = 8
BIG = 4096           # columns per x-tile pair (two 2 MiB DMAs)
NBIG = S // BIG      # 16
SUB = 512            # matmul free-dim tile (one PSUM bank, fp32 max)
QUAD = 4 * SUB       # 4 sub-tiles share one packed reduce
NQUAD = S // QUAD    # 32


def _host_tables(seed_emb, Wq, Wk, Wo, rope_freqs):
    """Fold q/RoPE/Wo/Wk into G[c] ([ES, 8]) and build the t table ([8, S])."""
    q = (seed_emb @ Wq).reshape(C, NH, HD)

    # u[c,h,s,:] = R(pos_s)^T q[c,h] is linear in
    # t(s) = [cos(y f0), sin(y f0), cos(y f1), sin(y f1),
    #         cos(x f0), sin(x f0), cos(x f1), sin(x f1)]
    U = np.zeros((C, NH, HD, 8), np.float32)
    for p in range(2):
        U[:, :, 2 * p, 2 * p] = q[:, :, 2 * p]
        U[:, :, 2 * p, 2 * p + 1] = q[:, :, 2 * p + 1]
        U[:, :, 2 * p + 1, 2 * p] = q[:, :, 2 * p + 1]
        U[:, :, 2 * p + 1, 2 * p + 1] = -q[:, :, 2 * p]
        U[:, :, 4 + 2 * p, 4 + 2 * p] = q[:, :, 4 + 2 * p]
        U[:, :, 4 + 2 * p, 4 + 2 * p + 1] = q[:, :, 4 + 2 * p + 1]
        U[:, :, 4 + 2 * p + 1, 4 + 2 * p] = q[:, :, 4 + 2 * p + 1]
        U[:, :, 4 + 2 * p + 1, 4 + 2 * p + 1] = -q[:, :, 4 + 2 * p]

    M = (Wo[:, 0][None, :, None, None] * U / np.sqrt(HD)).reshape(C, NH * HD, 8)
    G = np.einsum('ek,ckj->cej', Wk.astype(np.float32), M).astype(np.float32)

    ys = np.arange(H, dtype=np.float32) - H // 2
    xs = np.arange(W, dtype=np.float32) - W // 2
    ay = ys[:, None] * rope_freqs[None, :]
    ax = xs[:, None] * rope_freqs[None, :]
    ty = np.stack([np.cos(ay[:, 0]), np.sin(ay[:, 0]),
                   np.cos(ay[:, 1]), np.sin(ay[:, 1])], 0).astype(np.float32)
    tx = np.stack([np.cos(ax[:, 0]), np.sin(ax[:, 0]),
                   np.cos(ax[:, 1]), np.sin(ax[:, 1])], 0).astype(np.float32)
    t = np.empty((8, S), np.float32)
    t[:4] = np.repeat(ty, W, axis=1)   # s = y*W + x: y varies slowly
    t[4:] = np.tile(tx, (1, H))
    return G, t


def _build_program():
    import concourse.mybir as mybir
    from concourse import bacc
    from concourse.tile import TileContext
    from contextlib import ExitStack

    f32 = mybir.dt.float32
    nc = bacc.Bacc("TRN2")

    # g packs [G half0 | G half1 | block-ones]: cols 0-7 and 8-15 are the two
    # 128-row halves of G; cols 16-19 are the packed-reduce weights (col q has
    # ones in rows [32q, 32q+8)).  One DMA = one semaphore for all constants.
    x_d = nc.declare_dram_parameter("x", [ES, S], f32, isOutput=False)
    g_d = nc.declare_dram_parameter("g", [128, 20], f32, isOutput=False)
    t_d = nc.declare_dram_parameter("t", [NBIG, 8, BIG], f32, isOutput=False)
    out_d = nc.declare_dram_parameter("out", [S // SUB, SUB], f32, isOutput=True)

    x_r = x_d[:].rearrange("(k p) s -> p k s", p=128)  # row = k*128 + p

    with ExitStack() as ctx:
        tc = ctx.enter_context(TileContext(nc))
        const = ctx.enter_context(tc.tile_pool(name="const", bufs=1))
        xpool = ctx.enter_context(tc.tile_pool(name="xpool", bufs=6))
        tpool = ctx.enter_context(tc.tile_pool(name="tpool", bufs=2))
        mpool = ctx.enter_context(tc.tile_pool(name="mpool", bufs=3))
        opool = ctx.enter_context(tc.tile_pool(name="opool", bufs=3))
        ps128p = ctx.enter_context(tc.tile_pool(name="ps128", bufs=5, space="PSUM"))
        ps4p = ctx.enter_context(tc.tile_pool(name="ps4", bufs=3, space="PSUM"))
        g_sb = const.tile([128, 20], f32)
        nc.sync.dma_start(out=g_sb, in_=g_d[:])

        # Sacrificial matmul: absorb the constant-DMA semaphore into PE's
        # observed clock so no loop matmul needs a second sync-wait.
        dmy = ps4p.tile([8, 8], f32, tag="ps4")
        nc.tensor.matmul(out=dmy, lhsT=g_sb[:, 0:8], rhs=g_sb[:, 0:8],
                         start=True, stop=True)

        # Pre-zero the m128 slots: rows outside [32q, 32q+8) are never
        # written and must stay finite (the block-ones lhsT zeros them out
        # of the reduce, but PSUM would turn NaN*0 into NaN).
        for _ in range(3):
            mz = mpool.tile([128, SUB], f32, tag="m128")
            nc.vector.memset(mz, 0.0)

        for b in range(NBIG):
            # Two separate per-K-half DMAs: 16 KiB contiguous chunks and
            # independent completion, so half-0 matmuls start early.
            # x on the Sync DGE ring only: its sequencer must never stall
            # behind a small DMA's slot wait, so t/out issue from ACT.
            xk0 = xpool.tile([128, BIG], f32, tag="xk")
            nc.sync.dma_start(out=xk0, in_=x_r[:, 0, b * BIG:(b + 1) * BIG])
            xk1 = xpool.tile([128, BIG], f32, tag="xk")
            nc.sync.dma_start(out=xk1, in_=x_r[:, 1, b * BIG:(b + 1) * BIG])
            tt = tpool.tile([8, BIG], f32)
            nc.gpsimd.dma_start(out=tt, in_=t_d[b])
            for qb in range(BIG // QUAD):          # quads within this x tile
                Q = b * (BIG // QUAD) + qb
                m128 = mpool.tile([128, SUB], f32, tag="m128")
                # 4 sub-tiles stream concurrently through distinct 32-column
                # groups of the PE array, sharing one PSUM bank.
                ps128 = ps128p.tile([128, SUB], f32)
                for i in range(4):                 # sub-tiles within the quad
                    col = qb * QUAD + i * SUB
                    sl = slice(col, col + SUB)
                    nc.tensor.matmul(out=ps128[32 * i:32 * i + 8, :],
                                     lhsT=g_sb[:, 0:8], rhs=xk0[:, sl],
                                     start=True, stop=False,
                                     tile_position=(0, 32 * i))
                    nc.tensor.matmul(out=ps128[32 * i:32 * i + 8, :],
                                     lhsT=g_sb[:, 8:16], rhs=xk1[:, sl],
                                     start=False, stop=True,
                                     tile_position=(0, 32 * i))
                for i in range(4):
                    sl = slice(qb * QUAD + i * SUB, qb * QUAD + (i + 1) * SUB)
                    nc.vector.tensor_mul(out=m128[32 * i:32 * i + 8, :],
                                         in0=ps128[32 * i:32 * i + 8, :],
                                         in1=tt[:, sl])
                ps4 = ps4p.tile([4, SUB], f32, tag="ps4")
                nc.tensor.matmul(out=ps4, lhsT=g_sb[:, 16:20], rhs=m128,
                                 start=True, stop=True)
                ob = opool.tile([4, SUB], f32)
                nc.scalar.copy(out=ob, in_=ps4)    # ACT: keep DVE for muls
                nc.scalar.dma_start(out=out_d[4 * Q:4 * Q + 4, :], in_=ob)

    nc.finalize()  # runs Bacc.compile(): reg alloc + sync-wait legalization
    return nc


_PROGRAM = None


def kernel(x, seed_emb, Wq, Wk, Wo, bo, rope_freqs):
    from concourse.bass_utils import run_bass_kernel_spmd

    global _PROGRAM
    x = np.ascontiguousarray(np.asarray(x, np.float32))
    seed_emb = np.asarray(seed_emb, np.float32)
    Wq = np.asarray(Wq, np.float32)
    Wk = np.asarray(Wk, np.float32)
    Wo = np.asarray(Wo, np.float32)
    bo = np.asarray(bo, np.float32)
    rope_freqs = np.asarray(rope_freqs, np.float32)

    G, t = _host_tables(seed_emb, Wq, Wk, Wo, rope_freqs)
    t_tiled = np.ascontiguousarray(
        t.reshape(8, NBIG, BIG).transpose(1, 0, 2))          # [NBIG, 8, BIG]

    bones = np.zeros((128, 4), np.float32)
    for q in range(4):
        bones[32 * q:32 * q + 8, q] = 1.0

    if _PROGRAM is None:
        _PROGRAM = _build_program()
    nc = _PROGRAM

    in_maps = []
    for c in range(NCORES):
        gp = np.empty((128, 20), np.float32)
        gp[:, :16] = G[c].reshape(2, 128, 8).transpose(1, 0, 2).reshape(128, 16)
        gp[:, 16:20] = bones
        in_maps.append({
            "x": x[c].reshape(ES, S),
            "g": gp,
            "t": t_tiled,
        })

    res = run_bass_kernel_spmd(nc, in_maps, core_ids=list(range(NCORES)))
    out = np.empty((C, S), np.float32)
    for c in range(NCORES):
        out[c] = res.results[c]["out"].reshape(S)
    out += bo[0]
    return out.reshape(C * S, 1)


# revision 34
# speedup vs baseline: 1.0411x; 1.0411x over previous
"""Trainium2 Bass kernel for nn_AttentionPixelClassifier.

Math: the query sits at the RoPE origin, so its rotation is the identity and
every downstream op is linear in the trig values of the key positions.  The
whole pipeline (k-projection, RoPE, q.kT, head-combining linear) collapses to

    logits[c, s] = sum_j t[j, s] * (G[c]^T x[c])[j, s] + bo

where G[c] = Wk @ M[c] is a per-instance [ES, 8] matrix folded on the host
from (q[c], Wo, Wk), and t is an [8, S] table of cos/sin terms of the pixel
positions.  On-device work per core (one instance per core, data-parallel
over C): a [256 -> 8] matmul over all 65536 pixels, an elementwise multiply
by t, and an 8-row reduction.  The kernel is HBM-bound: 64 MiB of x per core.

Device schedule: 512-column sub-tiles, grouped in quads.  Each sub-tile u
does two K=128 matmuls (G halves) into a [8, 512] PSUM bank, then a DVE
multiply by t writes the product into rows [32q, 32q+8) of a shared
[128, 512] SBUF tile (32-aligned partition bases are required by BIR).  One
K=128 block-ones matmul then reduces all four sub-tiles at once into a
[4, 512] PSUM tile, which is copied (DVE, to keep matmul sync-waits
mergeable) and DMA'd out.  fp32 matmuls stream at ~2 cycles/column, so the
packed reduce keeps TensorE off the critical path; HBM DMA is the floor.
"""

import numpy as np

C, ES, H, W = 8, 256, 256, 256
NH, HD = 4, 8
S = H * W            # 65536 pixels
NCORES = 8
BIG = 2048           # columns per x DMA (one 1 MiB transfer)
NBIG = S // BIG      # 16
SUB = 512            # matmul free-dim tile (one PSUM bank, fp32 max)
QUAD = 4 * SUB       # 4 sub-tiles share one packed reduce
NQUAD = S // QUAD    # 32


def _host_tables(seed_emb, Wq, Wk, Wo, rope_freqs):
    """Fold q/RoPE/Wo/Wk into G[c] ([ES, 8]) and build the t table ([8, S])."""
    q = (seed_emb @ Wq).reshape(C, NH, HD)

    # u[c,h,s,:] = R(pos_s)^T q[c,h] is linear in
    # t(s) = [cos(y f0), sin(y f0), cos(y f1), sin(y f1),
    #         cos(x f0), sin(x f0), cos(x f1), sin(x f1)]
    U = np.zeros((C, NH, HD, 8), np.float32)
    for p in range(2):
        U[:, :, 2 * p, 2 * p] = q[:, :, 2 * p]
        U[:, :, 2 * p, 2 * p + 1] = q[:, :, 2 * p + 1]
        U[:, :, 2 * p + 1, 2 * p] = q[:, :, 2 * p + 1]
        U[:, :, 2 * p + 1, 2 * p + 1] = -q[:, :, 2 * p]
        U[:, :, 4 + 2 * p, 4 + 2 * p] = q[:, :, 4 + 2 * p]
        U[:, :, 4 + 2 * p, 4 + 2 * p + 1] = q[:, :, 4 + 2 * p + 1]
        U[:, :, 4 + 2 * p + 1, 4 + 2 * p] = q[:, :, 4 + 2 * p + 1]
        U[:, :, 4 + 2 * p + 1, 4 + 2 * p + 1] = -q[:, :, 4 + 2 * p]

    M = (Wo[:, 0][None, :, None, None] * U / np.sqrt(HD)).reshape(C, NH * HD, 8)
    G = np.einsum('ek,ckj->cej', Wk.astype(np.float32), M).astype(np.float32)

    ys = np.arange(H, dtype=np.float32) - H // 2
    xs = np.arange(W, dtype=np.float32) - W // 2
    ay = ys[:, None] * rope_freqs[None, :]
    ax = xs[:, None] * rope_freqs[None, :]
    ty = np.stack([np.cos(ay[:, 0]), np.sin(ay[:, 0]),
                   np.cos(ay[:, 1]), np.sin(ay[:, 1])], 0).astype(np.float32)
    tx = np.stack([np.cos(ax[:, 0]), np.sin(ax[:, 0]),
                   np.cos(ax[:, 1]), np.sin(ax[:, 1])], 0).astype(np.float32)
    t = np.empty((8, S), np.float32)
    t[:4] = np.repeat(ty, W, axis=1)   # s = y*W + x: y varies slowly
    t[4:] = np.tile(tx, (1, H))
    return G, t


def _build_program():
    import concourse.bass as bass
    import concourse.mybir as mybir
    from concourse import bacc
    from concourse.tile import TileContext
    from contextlib import ExitStack

    f32 = mybir.dt.float32
    nc = bacc.Bacc("TRN2")

    # g packs [G half0 | G half1 | block-ones]: cols 0-7 and 8-15 are the two
    # 128-row halves of G; cols 16-19 are the packed-reduce weights (col q has
    # ones in rows [32q, 32q+8)).  One DMA = one semaphore for all constants.
    x_d = nc.declare_dram_parameter("x", [ES, S], f32, isOutput=False)
    g_d = nc.declare_dram_parameter("g", [128, 20], f32, isOutput=False)
    tx_d = nc.declare_dram_parameter("tx", [4, BIG], f32, isOutput=False)
    ty_d = nc.declare_dram_parameter("ty", [NBIG, 4, 8], f32, isOutput=False)
    out_d = nc.declare_dram_parameter("out", [S // SUB, SUB], f32, isOutput=True)

    x_r = x_d[:].rearrange("(k p) s -> p k s", p=128)  # row = k*128 + p

    with ExitStack() as ctx:
        tc = ctx.enter_context(TileContext(nc))
        const = ctx.enter_context(tc.tile_pool(name="const", bufs=1))
        xpool = ctx.enter_context(tc.tile_pool(name="xpool", bufs=10))
        tpool = ctx.enter_context(tc.tile_pool(name="tpool", bufs=2))
        mpool = ctx.enter_context(tc.tile_pool(name="mpool", bufs=4))
        opool = ctx.enter_context(tc.tile_pool(name="opool", bufs=6))
        ps128p = ctx.enter_context(tc.tile_pool(name="ps128", bufs=5, space="PSUM"))
        ps4p = ctx.enter_context(tc.tile_pool(name="ps4", bufs=3, space="PSUM"))
        g_sb = const.tile([128, 20], f32)
        nc.sync.dma_start(out=g_sb, in_=g_d[:])

        # Sacrificial matmul: absorb the constant-DMA semaphore into PE's
        # observed clock so no loop matmul needs a second sync-wait.
        dmy = ps4p.tile([8, 8], f32, tag="ps4")
        nc.tensor.matmul(out=dmy, lhsT=g_sb[:, 0:8], rhs=g_sb[:, 0:8],
                         start=True, stop=True)

        typool = ctx.enter_context(tc.tile_pool(name="typool", bufs=4))
        for _ in range(2):
            tts = tpool.tile([8, BIG], f32, tag="tt")
            nc.gpsimd.dma_start(out=tts[4:8, :], in_=tx_d[:])

        # Pre-zero the m128 slots: rows outside [32q, 32q+8) are never
        # written and must stay finite (the block-ones lhsT zeros them out
        # of the reduce, but PSUM would turn NaN*0 into NaN).
        for _ in range(4):
            mz = mpool.tile([128, SUB], f32, tag="m128")
            nc.vector.memset(mz, 0.0)

        for b in range(NBIG):
            xt = xpool.tile([128, 2, BIG], f32)
            # x on the Sync DGE ring only: its sequencer must never stall
            # behind a small DMA's slot wait, so everything else issues
            # from the ACT / GpSimd rings.
            nc.sync.dma_start(out=xt, in_=x_r[:, :, b * BIG:(b + 1) * BIG])
            # t tile: x-part rows (4-7) were preloaded once (256-periodic
            # pattern); y-part rows (0-3) are piecewise-constant over image
            # rows, broadcast-expanded on ACT from a [4, 8] table.
            tt = tpool.tile([8, BIG], f32, tag="tt")
            ty = typool.tile([4, 8], f32)
            nc.gpsimd.dma_start(out=ty, in_=ty_d[b])
            ty_b = bass.AP(tensor=ty.tensor, offset=ty.offset,
                           ap=[ty.ap[0], ty.ap[1], [0, W]])
            nc.scalar.copy(out=tt[0:4, :].rearrange("p (r c) -> p r c", c=W),
                           in_=ty_b)
            for qb in range(BIG // QUAD):          # quads within this x tile
                Q = b * (BIG // QUAD) + qb
                m128 = mpool.tile([128, SUB], f32, tag="m128")
                # 4 sub-tiles stream concurrently through distinct 32-column
                # groups of the PE array, sharing one PSUM bank.
                ps128 = ps128p.tile([128, SUB], f32)
                for i in range(4):                 # sub-tiles within the quad
                    col = qb * QUAD + i * SUB
                    sl = slice(col, col + SUB)
                    nc.tensor.matmul(out=ps128[32 * i:32 * i + 8, :],
                                     lhsT=g_sb[:, 0:8], rhs=xt[:, 0, sl],
                                     start=True, stop=False,
                                     tile_position=(0, 32 * i))
                    nc.tensor.matmul(out=ps128[32 * i:32 * i + 8, :],
                                     lhsT=g_sb[:, 8:16], rhs=xt[:, 1, sl],
                                     start=False, stop=True,
                                     tile_position=(0, 32 * i))
                for i in range(4):
                    sl = slice(qb * QUAD + i * SUB, qb * QUAD + (i + 1) * SUB)
                    nc.vector.tensor_mul(out=m128[32 * i:32 * i + 8, :],
                                         in0=ps128[32 * i:32 * i + 8, :],
                                         in1=tt[:, sl])
                ps4 = ps4p.tile([4, SUB], f32, tag="ps4")
                nc.tensor.matmul(out=ps4, lhsT=g_sb[:, 16:20], rhs=m128,
                                 start=True, stop=True)
                ob = opool.tile([4, SUB], f32)
                nc.scalar.copy(out=ob, in_=ps4)    # ACT: keep DVE for muls
                nc.scalar.dma_start(out=out_d[4 * Q:4 * Q + 4, :], in_=ob)

    nc.finalize()  # runs Bacc.compile(): reg alloc + sync-wait legalization
    return nc


_PROGRAM = None


def kernel(x, seed_emb, Wq, Wk, Wo, bo, rope_freqs):
    from concourse.bass_utils import run_bass_kernel_spmd

    global _PROGRAM
    x = np.ascontiguousarray(np.asarray(x, np.float32))
    seed_emb = np.asarray(seed_emb, np.float32)
    Wq = np.asarray(Wq, np.float32)
    Wk = np.asarray(Wk, np.float32)
    Wo = np.asarray(Wo, np.float32)
    bo = np.asarray(bo, np.float32)
    rope_freqs = np.asarray(rope_freqs, np.float32)

    G, t = _host_tables(seed_emb, Wq, Wk, Wo, rope_freqs)
    tx_pat = np.ascontiguousarray(t[4:8, :BIG])            # [4, BIG] 256-periodic
    rows_per_big = BIG // W
    ty_tab = np.ascontiguousarray(
        t[0:4].reshape(4, NBIG, rows_per_big, W)[:, :, :, 0]
        .transpose(1, 0, 2))                                 # [NBIG, 4, 8]

    bones = np.zeros((128, 4), np.float32)
    for q in range(4):
        bones[32 * q:32 * q + 8, q] = 1.0

    if _PROGRAM is None:
        _PROGRAM = _build_program()
    nc = _PROGRAM

    in_maps = []
    for c in range(NCORES):
        gp = np.empty((128, 20), np.float32)
        gp[:, :16] = G[c].reshape(2, 128, 8).transpose(1, 0, 2).reshape(128, 16)
        gp[:, 16:20] = bones
        in_maps.append({
            "x": x[c].reshape(ES, S),
            "g": gp,
            "tx": tx_pat,
            "ty": ty_tab,
        })

    res = run_bass_kernel_spmd(nc, in_maps, core_ids=list(range(NCORES)))
    out = np.empty((C, S), np.float32)
    for c in range(NCORES):
        out[c] = res.results[c]["out"].reshape(S)
    out += bo[0]
    return out.reshape(C * S, 1)
